# revision 1
# baseline (speedup 1.0000x reference)
"""Locally-connected conv (per-location weights) + ReLU on 8 Trainium2 cores.

Problem: x (B=64, Cin=64, H=64, W=64), weights (H, W, Cout=64, Cin=64, 3, 3)
  out[r,a,i,j] = relu( sum_{b,c,d} weights[i,j,a,b,c,d] * xpad[r,b,i+c,j+d] )

Sharding: data-parallel over H — core cid owns output rows i in [8*cid, 8*cid+8).
No collectives; pure SPMD with per-core input slices.

Device strategy (per core):
  - Host pre-packs weights into contraction-major tiles so every DMA has
    multi-KB contiguous partition lines (full HBM bandwidth).
  - x is padded/transposed on host to x_t[b, u, r, v] (u=h+1, v=w+1 padded
    planes); pairs of planes are stacked into 128-partition SBUF tiles so a
    single K=128 matmul contracts Cin x 2 vertical taps at once.
  - Per output row-pair and 16-column block: 3 dual-tap (K=128) + pairs of
    single-tap (K=64, opposite partition halves, run concurrently on the PE
    via row-group tiling) matmuls per location accumulate into PSUM.
  - One PSUM bank holds 8 locations; a single start=True on the first matmul
    clears the bank's has_written bits, later matmuls self-initialize their
    region (overwrite-where-unset, accumulate-where-set).
  - ScalarE applies ReLU PSUM->SBUF; out streams back as ot[i, a, j, r].
"""

import ml_dtypes
import numpy as np

import concourse.bass as bass
import concourse.mybir as mybir
import concourse.tile as tile
from concourse import bacc
from concourse.bass_utils import run_bass_kernel_spmd

B = 64          # batch (= matmul N)
CIN = 64        # in channels
COUT = 64       # out channels (= matmul M)
H = 64
W = 64
KS = 3          # conv kernel size
NCORES = 8
RPC = H // NCORES        # output rows per core = 8
NPAIR = RPC // 2         # row pairs per core = 4
NPLANES = RPC + 2        # padded input planes per core = 10
NXP = NPLANES // 2       # paired x tiles = 5
WPAD = W + 2             # 66
NJQ = 4                  # j quarter-blocks
JQ = W // NJQ            # 16 columns per block
FP32 = mybir.dt.float32
# bf16 inputs + fp32 PSUM accumulation: 4x PE throughput and half the HBM
# traffic vs fp32 (fp32 matmul lowers to 2 half-speed passes). Measured
# end-to-end max rel err ~2.5e-3.
CDT = mybir.dt.bfloat16
NP_CDT = ml_dtypes.bfloat16

_PROGRAM = None
LAST_RESULTS = None


def _build_program():
    """One Bass program, SPMD across 8 cores (inputs differ per core)."""
    nc = bacc.Bacc("TRN2", target_bir_lowering=False, debug=False,
                   num_devices=NCORES)
    # wt[t, jq, k(128), d(3), kind(3), j16, a] — see _pack_weights for k/kind.
    wt = nc.dram_tensor("wt", [NPAIR, NJQ, 128, KS, KS, JQ, COUT], CDT,
                        kind="ExternalInput")
    # xt[plane(10), b, v, r] — padded x planes for this core's rows.
    xt = nc.dram_tensor("xt", [NPLANES, CIN, WPAD, B], CDT,
                        kind="ExternalInput")
    # ot[il, a, j, r]
    ot = nc.dram_tensor("ot", [RPC, COUT, W, B], FP32, kind="ExternalOutput")

    with tile.TileContext(nc) as tc:
        with (
            tc.tile_pool(name="xpool", bufs=1) as xpool,
            tc.tile_pool(name="wpool", bufs=2) as wpool,
            tc.tile_pool(name="opool", bufs=2) as opool,
            tc.tile_pool(name="pspool", bufs=2,
                         space=bass.MemorySpace.PSUM) as pspool,
        ):
            # All x planes stay resident: 5 tiles [128=(plane parity, b), r, v].
            xp = []
            for s in range(NXP):
                # [128, v, r]: matmul rhs xp[:, v, :] streams contiguous columns
                t = xpool.tile([128, WPAD, B], CDT, tag=f"xp{s}")
                nc.sync.dma_start(
                    t[:], xt[2 * s:2 * s + 2].rearrange("p b v r -> (p b) v r"))
                xp.append(t)

            for tp in range(NPAIR):          # row pair: rows il = 2tp, 2tp+1
                for jq in range(NJQ):
                    wtile = wpool.tile([128, KS, KS, JQ, COUT], CDT, tag="w")
                    nc.sync.dma_start(wtile[:], wt[tp, jq])
                    o0 = opool.tile([COUT, JQ, B], FP32, tag="o0")
                    o1 = opool.tile([COUT, JQ, B], FP32, tag="o1")
                    for jb in range(2):      # 8-column PSUM banks
                        # Each output row accumulates in TWO banks — one per
                        # PE row-group — so all K=64 matmuls on row-group 0
                        # run concurrently with the ones on row-group 64.
                        ps0a = pspool.tile([COUT, 8, B], FP32, tag="ps0a")
                        ps0b = pspool.tile([COUT, 8, B], FP32, tag="ps0b")
                        ps1a = pspool.tile([COUT, 8, B], FP32, tag="ps1a")
                        ps1b = pspool.tile([COUT, 8, B], FP32, tag="ps1b")
                        for d in range(KS):
                            for jj in range(8):
                                jl = jb * 8 + jj          # index into wtile j16
                                j = jq * JQ + jl          # global column
                                v = j + d                 # padded x column
                                first = (d == 0 and jj == 0)
                                last = (d == KS - 1 and jj == 7)
                                # row 2tp: c=0 (plane 2tp, rows 0-63 of xp[tp])
                                nc.tensor.matmul(
                                    ps0a[:, jj, :], wtile[0:64, d, 0, jl, :],
                                    xp[tp][0:64, v, :],
                                    start=first, stop=False)
                                # row 2tp: c=1 (plane 2tp+1, rows 64-127)
                                nc.tensor.matmul(
                                    ps0b[:, jj, :], wtile[64:128, d, 0, jl, :],
                                    xp[tp][64:128, v, :],
                                    start=first, stop=last)
                                # row 2tp+1: c=1 (plane 2tp+2, rows 0-63)
                                nc.tensor.matmul(
                                    ps1a[:, jj, :], wtile[0:64, d, 1, jl, :],
                                    xp[tp + 1][0:64, v, :],
                                    start=first, stop=last)
                                # row 2tp+1: c=2 (plane 2tp+3, rows 64-127)
                                nc.tensor.matmul(
                                    ps1b[:, jj, :], wtile[64:128, d, 1, jl, :],
                                    xp[tp + 1][64:128, v, :],
                                    start=first, stop=False)
                                # row 2tp single c=2: plane 2tp+2 = upper xp[tp+1]
                                nc.tensor.matmul(
                                    ps0a[:, jj, :], wtile[0:64, d, 2, jl, :],
                                    xp[tp + 1][0:64, v, :],
                                    start=False, stop=last)
                                # row 2tp+1 single c=0: plane 2tp+1 = lower xp[tp]
                                nc.tensor.matmul(
                                    ps1b[:, jj, :], wtile[64:128, d, 2, jl, :],
                                    xp[tp][64:128, v, :],
                                    start=False, stop=last)
                        # TensorTensor may read only ONE input from PSUM:
                        # ACT copies bank a, DVE adds bank b, ACT applies ReLU.
                        ob = jb * 8
                        s0 = o0[:, ob:ob + 8, :]
                        s1 = o1[:, ob:ob + 8, :]
                        nc.scalar.activation(
                            s0, ps0a[:], mybir.ActivationFunctionType.Copy)
                        nc.scalar.activation(
                            s1, ps1a[:], mybir.ActivationFunctionType.Copy)
                        nc.vector.tensor_add(s0, s0, ps0b[:])
                        nc.vector.tensor_add(s1, s1, ps1b[:])
                        nc.scalar.activation(
                            s0, s0, mybir.ActivationFunctionType.Relu)
                        nc.scalar.activation(
                            s1, s1, mybir.ActivationFunctionType.Relu)
                    nc.sync.dma_start(ot[2 * tp, :, jq * JQ:(jq + 1) * JQ, :], o0[:])
                    nc.sync.dma_start(ot[2 * tp + 1, :, jq * JQ:(jq + 1) * JQ, :], o1[:])
    nc.compile()
    return nc


def _pack_weights(weights):
    """weights (i, j, a, b, c, d) -> WH[T, jq, k, d, kind, j16, a] per row pair.

    kind 0 (row 2T duals):   k = c*64+b, c in {0,1}
    kind 1 (row 2T+1 duals): k = (c-1)*64+b, c in {1,2}
    kind 2 (singles):        k<64: (row 2T, c=2); k>=64: (row 2T+1, c=0)
    """
    wt6 = weights.transpose(0, 5, 4, 3, 1, 2)  # [i, d, c, b, j, a]
    even = wt6[0::2]                           # [32, d, c, b, j, a]
    odd = wt6[1::2]

    def stack_k(arr):  # [32, 3(d), 2(c), 64(b), 64(j), 64(a)] -> k-major
        a = arr.transpose(0, 2, 3, 1, 4, 5)    # [32, c, b, d, j, a]
        a = a.reshape(H // 2, 128, KS, NJQ, JQ, COUT)  # j -> (jq, j16)
        return a.transpose(0, 3, 1, 2, 4, 5)   # [32, jq, k, d, j16, a]

    d0 = stack_k(even[:, :, 0:2])
    d1 = stack_k(odd[:, :, 1:3])
    s = stack_k(np.concatenate([even[:, :, 2:3], odd[:, :, 0:1]], axis=2))
    # -> [32, jq, k, d, kind, j16, a]
    return np.ascontiguousarray(np.stack([d0, d1, s], axis=4))


def _prep_x(x):
    xpad = np.pad(x, ((0, 0), (0, 0), (1, 1), (1, 1)))
    return np.ascontiguousarray(xpad.transpose(2, 1, 3, 0))  # [u, b, v, r]


def kernel(x, weights):
    global _PROGRAM, LAST_RESULTS
    x = np.ascontiguousarray(np.asarray(x, dtype=np.float32))
    weights = np.ascontiguousarray(np.asarray(weights, dtype=np.float32))
    assert x.shape == (B, CIN, H, W) and weights.shape == (H, W, COUT, CIN, KS, KS)

    x_t = _prep_x(x)
    wh = _pack_weights(weights)                             # [32, jq, k, d, e, j16, a]

    wh = wh.astype(NP_CDT)
    x_t = x_t.astype(NP_CDT)
    in_maps = []
    for cid in range(NCORES):
        in_maps.append({
            "wt": np.ascontiguousarray(wh[4 * cid:4 * cid + 4]),
            "xt": np.ascontiguousarray(x_t[RPC * cid:RPC * cid + NPLANES]),
        })

    if _PROGRAM is None:
        _PROGRAM = _build_program()
    res = run_bass_kernel_spmd(_PROGRAM, in_maps, list(range(NCORES)))
    LAST_RESULTS = res

    # ot[il, a, j, r] per core -> out[r, a, i, j]
    full = np.concatenate([res.results[c]["ot"] for c in range(NCORES)], axis=0)
    return np.ascontiguousarray(full.transpose(3, 1, 0, 2))



# revision 11
# speedup vs baseline: 1.2479x; 1.2479x over previous
"""Locally-connected conv (per-location weights) + ReLU on 8 Trainium2 cores.

Problem: x (B=64, Cin=64, H=64, W=64), weights (H, W, Cout=64, Cin=64, 3, 3)
  out[r,a,i,j] = relu( sum_{b,c,d} weights[i,j,a,b,c,d] * xpad[r,b,i+c,j+d] )

Sharding: data-parallel over H — core cid owns output rows i in [8*cid, 8*cid+8).
No collectives; pure SPMD with per-core input slices.

Per-core design (v2 — fp8 weights, M=128 matmuls):
  - Weights are the dominant HBM traffic (604MB fp32 total). They are
    host-quantized to fp8 E3M4 (scale 64, folded back by pre-scaling x by
    1/64 — both exact exponent shifts), halving weight DMA vs bf16 and
    enabling 4-elem/cycle fast weight load into the PE.
  - x planes stay resident in SBUF as bf16 pair-tiles xp[s] = planes
    (2s, 2s+1) stacked on the partition axis; a K=128 matmul contracts
    Cin x 2 vertical taps at once. Mixed-dtype matmul (fp8 stationary x
    bf16 moving) is supported by the PE (both upcast to FP22 internally).
  - M=128: each dual matmul computes TWO output rows' channels at once
    (row 2t-1 taps c=1,2 and row 2t taps c=0,1 share the xp[t] K-tile).
    PSUM tile P_t[128, 8, 64] = one bank holds the row pair; edge rows
    0/7 share bank P_0. Leftover taps (even rows c=2, odd rows c=0) are
    K=64 singles packed pairwise on opposite PE row-groups.
  - One ACT per bank applies ReLU PSUM->SBUF bf16; host upcasts to fp32.
  Per (j, d): 3 duals M128/K128 + 2 edge duals M64/K128 + 8 singles
  M64/K64 = 13 matmuls; 2496 per core.
"""

import ml_dtypes
import numpy as np

import concourse.bass as bass
import concourse.mybir as mybir
import concourse.tile as tile
from concourse import bacc
from concourse.bass_utils import run_bass_kernel_spmd

B = 64          # batch (= matmul N)
CIN = 64        # in channels
COUT = 64       # out channels
H = 64
W = 64
KS = 3          # conv kernel size
NCORES = 8
RPC = H // NCORES        # output rows per core = 8
NPLANES = RPC + 2        # padded input planes per core = 10
NXP = NPLANES // 2       # paired x tiles = 5
WPAD = W + 2             # 66
NBLK = 8                 # j blocks per core
JB = W // NBLK           # 8 columns per block
WCOLS = 768              # weight cols per (j, d): 3*128 duals + 2*64 edge + 4*64 singles
FP32 = mybir.dt.float32
BF16 = mybir.dt.bfloat16
FP8 = mybir.dt.float8e3          # E3M4: 4 mantissa bits
NP_FP8 = ml_dtypes.float8_e3m4
NP_BF16 = ml_dtypes.bfloat16
WSCALE = 64.0                    # w*64 in fp8, x/64 in bf16: exact shifts

# PSUM bank k holds rows (lo at partitions 0:64, hi at partitions 64:128)
BANK_ROWS = [(7, 0), (1, 2), (3, 4), (5, 6)]

_PROGRAM = None
LAST_RESULTS = None


def _build_program():
    nc = bacc.Bacc("TRN2", target_bir_lowering=False, debug=False,
                   num_devices=NCORES)
    # wt[blk, k(128), jj, d, col] — see _pack_weights for the col layout.
    wt = nc.dram_tensor("wt", [NBLK, 128, JB, KS, WCOLS], FP8,
                        kind="ExternalInput")
    # xt[plane(10), b, v, r] — padded x/64 planes for this core's rows.
    xt = nc.dram_tensor("xt", [NPLANES, CIN, WPAD, B], BF16,
                        kind="ExternalInput")
    # ot[blk, p(128), bank, jj, r]; partition p = hi/lo row half x channel
    ot = nc.dram_tensor("ot", [NBLK, 128, 4, JB, B], BF16,
                        kind="ExternalOutput")
    # zero weights: one M=128 dummy matmul per block starts bank 0's psum
    # accumulation group across all 128 partitions (rows 7/0 only ever get
    # M=64 writes, which the psum group tracker can't use as starters).
    zw = nc.dram_tensor("zw", [128, 128], FP8, kind="ExternalInput")

    with tile.TileContext(nc) as tc:
        with (
            tc.tile_pool(name="xpool", bufs=1) as xpool,
            tc.tile_pool(name="wpool", bufs=2) as wpool,
            tc.tile_pool(name="opool", bufs=2) as opool,
            tc.tile_pool(name="pspool", bufs=2,
                         space=bass.MemorySpace.PSUM) as pspool,
        ):
            # All x planes resident: 5 tiles [128=(plane pair, b), v, r].
            xp = []
            for s in range(NXP):
                t = xpool.tile([128, WPAD, B], BF16, tag=f"xp{s}")
                nc.sync.dma_start(
                    t[:], xt[2 * s:2 * s + 2].rearrange("p b v r -> (p b) v r"))
                xp.append(t)
            zt = xpool.tile([128, 128], FP8, tag="zt")
            nc.sync.dma_start(zt[:], zw[:])

            for blk in range(NBLK):
                wtile = wpool.tile([128, JB, KS, WCOLS], FP8, tag="w")
                nc.sync.dma_start(wtile[:], wt[blk])
                # 4 PSUM banks accumulate this block's 8 columns.
                P = [pspool.tile([128, JB, B], FP32, tag=f"ps{k}",
                                 name=f"ps{k}")
                     for k in range(4)]
                # start bank 0's group over all 128 partitions (writes zeros)
                nc.tensor.matmul(P[0][:, 0, :], zt[:], xp[0][:, 0, :],
                                 start=True, stop=False)
                for jj in range(JB):
                    j = blk * JB + jj
                    for d in range(KS):
                        v = j + d
                        first = (jj == 0 and d == 0)
                        last = (jj == JB - 1 and d == KS - 1)
                        wjd = wtile[:, jj, d]

                        def duals(stop):
                            # Duals: rows (2t-1, 2t) via xp[t]; M=128.
                            for t in (1, 2, 3):
                                nc.tensor.matmul(
                                    P[t][:, jj, :],
                                    wjd[:, (t - 1) * 128:t * 128],
                                    xp[t][:, v, :], start=first, stop=stop)

                        def rest():
                            # Edge rows: row 0 (c=0,1 via xp[0]) -> P0 hi;
                            # row 7 (c=1,2 via xp[4]) -> P0 lo. The psum
                            # group tracker mis-addresses partition-base-64
                            # outputs, so those skip it (data-path pending-
                            # zero semantics are still fully checked).
                            nc.tensor.matmul(
                                P[0][64:128, jj, :], wjd[:, 384:448],
                                xp[0][:, v, :], start=False, stop=False,
                                skip_group_check=True)
                            nc.tensor.matmul(
                                P[0][0:64, jj, :], wjd[:, 448:512],
                                xp[4][:, v, :], start=False, stop=False)
                            # Singles: even row 2s c=2 (lower xp[s+1]) and
                            # odd row 2s+1 c=0 (upper xp[s]) packed in one
                            # col-64 tile; opposite PE row-groups overlap.
                            for s in range(4):
                                o = 512 + 64 * s
                                # even row 2s = hi half of bank s
                                nc.tensor.matmul(
                                    P[s][64:128, jj, :], wjd[0:64, o:o + 64],
                                    xp[s + 1][0:64, v, :], start=False,
                                    stop=False, skip_group_check=True)
                                # odd row 2s+1 = lo half of bank (s+1)%4
                                nc.tensor.matmul(
                                    P[(s + 1) % 4][0:64, jj, :],
                                    wjd[64:128, o:o + 64],
                                    xp[s][64:128, v, :], start=False,
                                    stop=False)

                        if not last:
                            duals(False)
                            rest()
                        else:
                            # stops must come from full-128-partition
                            # instructions so the group tracker's clears
                            # cover both halves of each bank.
                            rest()
                            duals(True)
                            nc.tensor.matmul(
                                P[0][:, 0, :], zt[:], xp[0][:, 0, :],
                                start=False, stop=True)
                ob = opool.tile([128, 4, JB, B], BF16, tag="ob")
                for k in range(4):
                    nc.scalar.activation(
                        ob[:, k], P[k][:], mybir.ActivationFunctionType.Relu)
                nc.sync.dma_start(ot[blk], ob[:])
    nc.compile()
    return nc


def _pack_weights(wq):
    """wq fp8 (i, j, a, b, c, d) -> per-core [blk, 128, jj, d, WCOLS].

    Col layout per (j, d):
      [0:384)    D1..D3: dual t: cols (t-1)*128+[row 2t-1 a | row 2t a],
                 partition k = cc*64+b, cc indexing planes (2t, 2t+1):
                 row 2t-1 uses c=cc+1, row 2t uses c=cc.
      [384:448)  E0: row 0, k=(cc,b) ~ c=cc      (planes 0,1 = xp[0])
      [448:512)  E7: row 7, k=(cc,b) ~ c=cc+1    (planes 8,9 = xp[4])
      [512:768)  S_s (s=0..3): partitions 0:64 = row 2s c=2,
                 partitions 64:128 = row 2s+1 c=0.
    """
    wc = wq.reshape(NCORES, RPC, W, COUT, CIN, KS, KS)  # [cid,r,j,a,b,c,d]

    def kmaj(arr):  # [cid, j, a, b, cc, d] -> [cid, (cc b), j, d, a]
        return arr.transpose(0, 4, 3, 1, 5, 2).reshape(
            NCORES, 128, W, KS, COUT)

    def bmaj(arr):  # [cid, j, a, b, d] -> [cid, b, j, d, a]
        return arr.transpose(0, 3, 1, 4, 2)

    cols = []
    for t in (1, 2, 3):
        ca = kmaj(wc[:, 2 * t - 1, :, :, :, 1:3, :])   # row 2t-1, c=1,2
        cb = kmaj(wc[:, 2 * t, :, :, :, 0:2, :])       # row 2t,   c=0,1
        cols.append(np.concatenate([ca, cb], axis=-1))  # [cid,128,j,d,128]
    cols.append(kmaj(wc[:, 0, :, :, :, 0:2, :]))        # E0
    cols.append(kmaj(wc[:, 7, :, :, :, 1:3, :]))        # E7
    for s in range(4):
        lo = bmaj(wc[:, 2 * s, :, :, :, 2, :])          # [cid, b, j, d, a]
        hi = bmaj(wc[:, 2 * s + 1, :, :, :, 0, :])
        cols.append(np.concatenate([lo, hi], axis=1))   # [cid,128,j,d,64]
    full = np.concatenate(cols, axis=-1)                # [cid,128,64,3,768]
    # j -> (blk, jj); reorder to [cid, blk, 128, jj, d, col]
    full = full.reshape(NCORES, 128, NBLK, JB, KS, WCOLS)
    return np.ascontiguousarray(full.transpose(0, 2, 1, 3, 4, 5))


def _prep_x(x):
    xpad = np.pad(x * (1.0 / WSCALE), ((0, 0), (0, 0), (1, 1), (1, 1)))
    return np.ascontiguousarray(xpad.transpose(2, 1, 3, 0))  # [u, b, v, r]


def kernel(x, weights):
    global _PROGRAM, LAST_RESULTS
    x = np.ascontiguousarray(np.asarray(x, dtype=np.float32))
    weights = np.ascontiguousarray(np.asarray(weights, dtype=np.float32))
    assert x.shape == (B, CIN, H, W) and weights.shape == (H, W, COUT, CIN, KS, KS)

    x_t = _prep_x(x).astype(NP_BF16)
    wq = (weights * WSCALE).astype(NP_FP8)  # [i, j, a, b, c, d]
    wh = _pack_weights(wq)                  # [cid, blk, 128, jj, d, col]

    in_maps = []
    for cid in range(NCORES):
        in_maps.append({
            "wt": np.ascontiguousarray(wh[cid]),
            "xt": np.ascontiguousarray(x_t[RPC * cid:RPC * cid + NPLANES]),
            "zw": np.zeros((128, 128), dtype=NP_FP8),
        })

    if _PROGRAM is None:
        _PROGRAM = _build_program()
    res = run_bass_kernel_spmd(_PROGRAM, in_maps, list(range(NCORES)))
    LAST_RESULTS = res

    # ot[blk, p, bank, jj, r]: p = (half h)*64 + a; bank k rows per
    # BANK_ROWS (lo half h=0 -> row BANK_ROWS[k][0], hi h=1 -> [k][1]).
    out = np.empty((B, COUT, H, W), dtype=np.float32)
    for cid in range(NCORES):
        arr = np.asarray(res.results[cid]["ot"]).astype(np.float32)
        arr = arr.reshape(NBLK, 2, COUT, 4, JB, B)  # [blk, h, a, k, jj, r]
        for k in range(4):
            for h in range(2):
                i_local = BANK_ROWS[k][h]
                # [blk, a, jj, r] -> out[r, a, i, blk*JB+jj]
                out[:, :, RPC * cid + i_local, :] = (
                    arr[:, h, :, k, :, :].transpose(3, 1, 0, 2)
                    .reshape(B, COUT, W))
    return out


# revision 12
# speedup vs baseline: 1.6498x; 1.3220x over previous
"""Locally-connected conv (per-location weights) + ReLU on 8 Trainium2 cores.

Problem: x (B=64, Cin=64, H=64, W=64), weights (H, W, Cout=64, Cin=64, 3, 3)
  out[r,a,i,j] = relu( sum_{b,c,d} weights[i,j,a,b,c,d] * xpad[r,b,i+c,j+d] )

Sharding: data-parallel over H — core cid owns output rows i in [8*cid, 8*cid+8).
No collectives; pure SPMD with per-core input slices.

Per-core design (v2 — fp8 weights, M=128 matmuls):
  - Weights are the dominant HBM traffic (604MB fp32 total). They are
    host-quantized to fp8 E3M4 (scale 64, folded back by pre-scaling x by
    1/64 — both exact exponent shifts), halving weight DMA vs bf16 and
    enabling 4-elem/cycle fast weight load into the PE.
  - x planes stay resident in SBUF as bf16 pair-tiles xp[s] = planes
    (2s, 2s+1) stacked on the partition axis; a K=128 matmul contracts
    Cin x 2 vertical taps at once. Mixed-dtype matmul (fp8 stationary x
    bf16 moving) is supported by the PE (both upcast to FP22 internally).
  - M=128: each dual matmul computes TWO output rows' channels at once
    (row 2t-1 taps c=1,2 and row 2t taps c=0,1 share the xp[t] K-tile).
    PSUM tile P_t[128, 8, 64] = one bank holds the row pair; edge rows
    0/7 share bank P_0. Leftover taps (even rows c=2, odd rows c=0) are
    K=64 singles packed pairwise on opposite PE row-groups.
  - One ACT per bank applies ReLU PSUM->SBUF bf16; host upcasts to fp32.
  Per (j, d): 3 duals M128/K128 + 2 edge duals M64/K128 + 8 singles
  M64/K64 = 13 matmuls; 2496 per core.
"""

import ml_dtypes
import numpy as np

import concourse.bass as bass
import concourse.mybir as mybir
import concourse.tile as tile
from concourse import bacc
from concourse.bass_utils import run_bass_kernel_spmd

B = 64          # batch (= matmul N)
CIN = 64        # in channels
COUT = 64       # out channels
H = 64
W = 64
KS = 3          # conv kernel size
NCORES = 8
RPC = H // NCORES        # output rows per core = 8
NPLANES = RPC + 2        # padded input planes per core = 10
NXP = NPLANES // 2       # paired x tiles = 5
WPAD = W + 2             # 66
NBLK = 8                 # j blocks per core
JB = W // NBLK           # 8 columns per block
WCOLS = 768              # weight cols per (j, d): 3*128 duals + 2*64 edge + 4*64 singles
FP32 = mybir.dt.float32
BF16 = mybir.dt.bfloat16
FP8 = mybir.dt.float8e3          # E3M4: 4 mantissa bits
NP_FP8 = ml_dtypes.float8_e3m4
NP_BF16 = ml_dtypes.bfloat16
WSCALE = 64.0                    # w*64 in fp8, x/64 in bf16: exact shifts

# PSUM bank k holds rows (lo at partitions 0:64, hi at partitions 64:128)
BANK_ROWS = [(7, 0), (1, 2), (3, 4), (5, 6)]

_PROGRAM = None
LAST_RESULTS = None


def _build_program():
    nc = bacc.Bacc("TRN2", target_bir_lowering=False, debug=False,
                   num_devices=NCORES)
    # wt[blk, k(128), jj, d, col] — see _pack_weights for the col layout.
    wt = nc.dram_tensor("wt", [NBLK, 128, JB, KS, WCOLS], FP8,
                        kind="ExternalInput")
    # xt[plane(10), b, v, r] — padded x/64 planes for this core's rows.
    xt = nc.dram_tensor("xt", [NPLANES, CIN, WPAD, B], BF16,
                        kind="ExternalInput")
    # ot[blk, p(128), bank, jj, r]; partition p = hi/lo row half x channel
    ot = nc.dram_tensor("ot", [NBLK, 128, 4, JB, B], BF16,
                        kind="ExternalOutput")
    # zero weights: one M=128 dummy matmul per block starts bank 0's psum
    # accumulation group across all 128 partitions (rows 7/0 only ever get
    # M=64 writes, which the psum group tracker can't use as starters).
    zw = nc.dram_tensor("zw", [128, 128], FP8, kind="ExternalInput")

    with tile.TileContext(nc) as tc:
        with (
            tc.tile_pool(name="xpool", bufs=1) as xpool,
            tc.tile_pool(name="wpool", bufs=2) as wpool,
            tc.tile_pool(name="opool", bufs=2) as opool,
            tc.tile_pool(name="pspool", bufs=2,
                         space=bass.MemorySpace.PSUM) as pspool,
        ):
            # All x planes resident: 5 tiles [128=(plane pair, b), v, r].
            xp = []
            for s in range(NXP):
                t = xpool.tile([128, WPAD, B], BF16, tag=f"xp{s}")
                nc.sync.dma_start(
                    t[:], xt[2 * s:2 * s + 2].rearrange("p b v r -> (p b) v r"))
                xp.append(t)
            zt = xpool.tile([128, 128], FP8, tag="zt")
            nc.sync.dma_start(zt[:], zw[:])

            for blk in range(NBLK):
                wtile = wpool.tile([128, JB, KS, WCOLS], FP8, tag="w")
                nc.sync.dma_start(wtile[:], wt[blk])
                # 4 PSUM banks accumulate this block's 8 columns.
                P = [pspool.tile([128, JB, B], FP32, tag=f"ps{k}",
                                 name=f"ps{k}")
                     for k in range(4)]
                # start bank 0's group over all 128 partitions (writes zeros)
                nc.tensor.matmul(P[0][:, 0, :], zt[:], xp[0][:, 0, :],
                                 start=True, stop=False)
                # Geometry-batched issue order: the PE pipelines same-shaped
                # matmuls back-to-back (fill of i+1 overlaps drain of i) but
                # pays a ~170ns drain stall at every array-geometry change.
                # Batching per block: 3 transitions instead of 3 per (jj,d).
                # Accumulation order within a bank is irrelevant.
                for t in (1, 2, 3):
                    # Duals: rows (2t-1, 2t) via xp[t]; M=128, K=128.
                    for jj in range(JB):
                        for d in range(KS):
                            v = blk * JB + jj + d
                            first = (jj == 0 and d == 0)
                            nc.tensor.matmul(
                                P[t][:, jj, :],
                                wtile[:, jj, d, (t - 1) * 128:t * 128],
                                xp[t][:, v, :], start=first, stop=False)
                for jj in range(JB):
                    for d in range(KS):
                        v = blk * JB + jj + d
                        wjd = wtile[:, jj, d]
                        # Edge rows: row 0 (c=0,1 via xp[0]) -> P0 hi; row 7
                        # (c=1,2 via xp[4]) -> P0 lo. Disjoint col groups,
                        # issued as a pair. The psum group tracker
                        # mis-addresses partition-base-64 outputs, so those
                        # skip it (pending-zero data checks still run).
                        nc.tensor.matmul(
                            P[0][64:128, jj, :], wjd[:, 384:448],
                            xp[0][:, v, :], start=False, stop=False,
                            skip_group_check=True)
                        nc.tensor.matmul(
                            P[0][0:64, jj, :], wjd[:, 448:512],
                            xp[4][:, v, :], start=False, stop=False)
                for jj in range(JB):
                    for d in range(KS):
                        v = blk * JB + jj + d
                        wjd = wtile[:, jj, d]
                        # Singles: even row 2s c=2 (lower xp[s+1]) and odd
                        # row 2s+1 c=0 (upper xp[s]) packed in one col-64
                        # tile; opposite PE row-groups run concurrently.
                        for s in range(4):
                            o = 512 + 64 * s
                            nc.tensor.matmul(
                                P[s][64:128, jj, :], wjd[0:64, o:o + 64],
                                xp[s + 1][0:64, v, :], start=False,
                                stop=False, skip_group_check=True)
                            nc.tensor.matmul(
                                P[(s + 1) % 4][0:64, jj, :],
                                wjd[64:128, o:o + 64],
                                xp[s][64:128, v, :], start=False,
                                stop=False)
                # closers: full-128-partition zero matmuls carry the stop
                # flags so the group tracker's clears cover whole banks.
                for k in range(4):
                    nc.tensor.matmul(P[k][:, 0, :], zt[:], xp[0][:, 0, :],
                                     start=False, stop=True)
                ob = opool.tile([128, 4, JB, B], BF16, tag="ob")
                for k in range(4):
                    nc.scalar.activation(
                        ob[:, k], P[k][:], mybir.ActivationFunctionType.Relu)
                nc.sync.dma_start(ot[blk], ob[:])
    nc.compile()
    return nc


def _pack_weights(wq):
    """wq fp8 (i, j, a, b, c, d) -> per-core [blk, 128, jj, d, WCOLS].

    Col layout per (j, d):
      [0:384)    D1..D3: dual t: cols (t-1)*128+[row 2t-1 a | row 2t a],
                 partition k = cc*64+b, cc indexing planes (2t, 2t+1):
                 row 2t-1 uses c=cc+1, row 2t uses c=cc.
      [384:448)  E0: row 0, k=(cc,b) ~ c=cc      (planes 0,1 = xp[0])
      [448:512)  E7: row 7, k=(cc,b) ~ c=cc+1    (planes 8,9 = xp[4])
      [512:768)  S_s (s=0..3): partitions 0:64 = row 2s c=2,
                 partitions 64:128 = row 2s+1 c=0.
    """
    wc = wq.reshape(NCORES, RPC, W, COUT, CIN, KS, KS)  # [cid,r,j,a,b,c,d]

    def kmaj(arr):  # [cid, j, a, b, cc, d] -> [cid, (cc b), j, d, a]
        return arr.transpose(0, 4, 3, 1, 5, 2).reshape(
            NCORES, 128, W, KS, COUT)

    def bmaj(arr):  # [cid, j, a, b, d] -> [cid, b, j, d, a]
        return arr.transpose(0, 3, 1, 4, 2)

    cols = []
    for t in (1, 2, 3):
        ca = kmaj(wc[:, 2 * t - 1, :, :, :, 1:3, :])   # row 2t-1, c=1,2
        cb = kmaj(wc[:, 2 * t, :, :, :, 0:2, :])       # row 2t,   c=0,1
        cols.append(np.concatenate([ca, cb], axis=-1))  # [cid,128,j,d,128]
    cols.append(kmaj(wc[:, 0, :, :, :, 0:2, :]))        # E0
    cols.append(kmaj(wc[:, 7, :, :, :, 1:3, :]))        # E7
    for s in range(4):
        lo = bmaj(wc[:, 2 * s, :, :, :, 2, :])          # [cid, b, j, d, a]
        hi = bmaj(wc[:, 2 * s + 1, :, :, :, 0, :])
        cols.append(np.concatenate([lo, hi], axis=1))   # [cid,128,j,d,64]
    full = np.concatenate(cols, axis=-1)                # [cid,128,64,3,768]
    # j -> (blk, jj); reorder to [cid, blk, 128, jj, d, col]
    full = full.reshape(NCORES, 128, NBLK, JB, KS, WCOLS)
    return np.ascontiguousarray(full.transpose(0, 2, 1, 3, 4, 5))


def _prep_x(x):
    xpad = np.pad(x * (1.0 / WSCALE), ((0, 0), (0, 0), (1, 1), (1, 1)))
    return np.ascontiguousarray(xpad.transpose(2, 1, 3, 0))  # [u, b, v, r]


def kernel(x, weights):
    global _PROGRAM, LAST_RESULTS
    x = np.ascontiguousarray(np.asarray(x, dtype=np.float32))
    weights = np.ascontiguousarray(np.asarray(weights, dtype=np.float32))
    assert x.shape == (B, CIN, H, W) and weights.shape == (H, W, COUT, CIN, KS, KS)

    x_t = _prep_x(x).astype(NP_BF16)
    wq = (weights * WSCALE).astype(NP_FP8)  # [i, j, a, b, c, d]
    wh = _pack_weights(wq)                  # [cid, blk, 128, jj, d, col]

    in_maps = []
    for cid in range(NCORES):
        in_maps.append({
            "wt": np.ascontiguousarray(wh[cid]),
            "xt": np.ascontiguousarray(x_t[RPC * cid:RPC * cid + NPLANES]),
            "zw": np.zeros((128, 128), dtype=NP_FP8),
        })

    if _PROGRAM is None:
        _PROGRAM = _build_program()
    res = run_bass_kernel_spmd(_PROGRAM, in_maps, list(range(NCORES)))
    LAST_RESULTS = res

    # ot[blk, p, bank, jj, r]: p = (half h)*64 + a; bank k rows per
    # BANK_ROWS (lo half h=0 -> row BANK_ROWS[k][0], hi h=1 -> [k][1]).
    out = np.empty((B, COUT, H, W), dtype=np.float32)
    for cid in range(NCORES):
        arr = np.asarray(res.results[cid]["ot"]).astype(np.float32)
        arr = arr.reshape(NBLK, 2, COUT, 4, JB, B)  # [blk, h, a, k, jj, r]
        for k in range(4):
            for h in range(2):
                i_local = BANK_ROWS[k][h]
                # [blk, a, jj, r] -> out[r, a, i, blk*JB+jj]
                out[:, :, RPC * cid + i_local, :] = (
                    arr[:, h, :, k, :, :].transpose(3, 1, 0, 2)
                    .reshape(B, COUT, W))
    return out
